# revision 26
# baseline (speedup 1.0000x reference)
"""Trainium2 Bass kernel for nn_Mlp_84275848282705 (SmoothQuant-style quantized ViT MLP).

v5: data-parallel over tokens (12608 = 8 x 1576).
- x SBUF-resident through quant (no re-read stream).
- w1 streamed twice in H-quarter column tiles: pass A feeds the wcol
  (per-channel absmax) reduces only; pass B (emitted per-quarter, ring)
  feeds s1 + quant. No full-row pass, no 9 MB residency.
- cs chain: y = round(0.5*log2(ratio) - 0.5); up-compare on squares.
- sx chain: both partition reduces in ONE PartitionAllReduce ([128,2]).
- s1 scale/recip on the tiny [128,6] column form (DMA bounce on GpSimd's
  queue); A1 and invs1_bc derive from it.
- s1 absmax: ACT Abs for q0/q1 (ACT is idle early), double signed-max
  STT chains on DVE for q2/q3 (ACT saturates with gelu later).
- Quarter tails q1..q3 are emitted inside the fc1 loop so the ACT queue
  interleaves abs with gelu instead of blocking gelu behind them.
- fc1 PSUM paired: one [128,1024] psum tile holds chunks 0+1 (two accum
  groups), one [128,576] holds chunks 2+3; gelu reads each pair in a
  single ACT op (halves gelu op count).
- w2 absmax stream: GpSimd squares (TT mult) + DVE max; sqrt on the
  tiny column form.
- w2 quant on DVE (TT mult + TS round): kt 0-11 between AR2 pack and
  unpack, kt 12-23 interleaved into fc2's first chunk.
- fc2 hq: round-trick split between ACT and DVE to balance both queues.
"""
import sys

sys.path.insert(0, "/opt/trn_rl_repo")

import numpy as np

B, N, C, H = 64, 197, 768, 3072
TOK = B * N             # 12608
N_CORES = 8
TLOC = TOK // N_CORES   # 1576
NRES = 9               # h tiles resident in SBUF (of 24); last 15 spill to DRAM
RND = 12582912.0        # 1.5*2^23: RNE integer-round magic const (valid for |x| <= 2^22)
EPS = 1e-8
INV_LN2 = 1.4426950408889634
LN2 = 0.6931471805599453
R127 = float(np.float32(1.0) / np.float32(127.0))
R255 = float(np.float32(1.0) / np.float32(255.0))
HQ = H // 4             # 768: H quarter


def _chunks(t_pad, step, t_loc):
    t_cov = ((t_loc + 63) // 64) * 64
    out, off = [], 0
    while off < t_cov:
        w = min(step, t_cov - off)
        out.append((off, w))
        off += w
    return out


def build(n_cores=N_CORES, t_loc=TLOC):
    import concourse.bacc as bacc
    import concourse.tile as tile
    from concourse import mybir

    F32 = mybir.dt.float32
    t_pad = ((t_loc + 127) // 128) * 128

    nc = bacc.Bacc("TRN2", target_bir_lowering=False, debug=False,
                   enable_asserts=False, num_devices=n_cores)

    io = dict(
        xT=nc.dram_tensor("xT", [C, t_pad], F32, kind="ExternalInput").ap(),
        w1T=nc.dram_tensor("w1T", [C, H], F32, kind="ExternalInput").ap(),
        w2T=nc.dram_tensor("w2T", [H, C], F32, kind="ExternalInput").ap(),
        b1=nc.dram_tensor("b1", [H], F32, kind="ExternalInput").ap(),
        b2=nc.dram_tensor("b2", [C], F32, kind="ExternalInput").ap(),
        # transposed output: [C, t_pad]; host transposes back
        out_e=nc.dram_tensor("out", [C, t_pad], F32, kind="ExternalOutput").ap(),
    )

    with tile.TileContext(nc) as tc:
        _emit(nc, tc, io, n_cores, t_loc, t_pad)
    nc.compile()
    return nc


def _emit(nc, tc, io, n_cores, t_loc, t_pad):
    from contextlib import ExitStack
    from concourse import mybir, bass_isa
    from concourse.tile import add_dep_helper

    F32 = mybir.dt.float32
    BF16 = mybir.dt.bfloat16
    AT = mybir.AluOpType
    AFT = mybir.ActivationFunctionType
    AX = mybir.AxisListType.X
    ROP = bass_isa.ReduceOp
    RG = [list(range(n_cores))]

    xT, w1T, w2T, b1, b2, out_e = (io[k] for k in
                                   ("xT", "w1T", "w2T", "b1", "b2", "out_e"))

    CH = _chunks(t_pad, 512, t_loc)
    t_cov = ((t_loc + 63) // 64) * 64  # 1600
    n_spill = 24 - NRES

    DVE, ACT, GPS, SYNC = nc.vector, nc.scalar, nc.gpsimd, nc.sync
    MM = nc.tensor.matmul

    with ExitStack() as ctx:
        const = ctx.enter_context(tc.tile_pool(name="const", bufs=1))
        dram = ctx.enter_context(tc.tile_pool(name="dram", bufs=1, space="DRAM"))
        psp = ctx.enter_context(tc.tile_pool(name="ps", bufs=2, space="PSUM"))
        psp2 = ctx.enter_context(tc.tile_pool(name="ps2", bufs=2, space="PSUM"))

        # DRAM scratch (collectives + layout bounces + h spill)
        st_in = dram.tile([128, 12], F32)
        st_out = dram.tile([128, 12], F32)
        sc_in = dram.tile([1, 8], F32)
        sc_out = dram.tile([1, 8], F32)
        sc_in2 = dram.tile([1, 8], F32)
        sc_out2 = dram.tile([1, 8], F32)
        s1row_d = dram.tile([1, H], F32)
        i1row_d = dram.tile([1, H], F32)
        s2row = dram.tile([1, C], F32)
        i2row = dram.tile([1, C], F32)
        hd = dram.tile([n_spill, 128, t_cov], F32)

        # small const tiles
        b1t = const.tile([128, 24], F32)
        SYNC.dma_start(out=b1t[:], in_=b1.rearrange("(k p) -> p k", p=128))
        b2t = const.tile([128, 6], F32)
        SYNC.dma_start(out=b2t[:], in_=b2.rearrange("(k p) -> p k", p=128))

        stats12 = const.tile([128, 12], F32)
        stat_max = stats12[:, 0:6]
        stat_nm = stats12[:, 6:12]
        stat_abs = const.tile([128, 6], F32)
        wcol = const.tile([128, 6], F32)
        wcol_t = const.tile([128, 24], F32)
        n_hcols = NRES + 2 * n_spill
        habs_cols = const.tile([128, n_hcols], F32)
        omax_cols = const.tile([128, 24], F32)
        onm_cols = const.tile([128, 24], F32)
        s1col = const.tile([128, 24], F32)
        i1col = const.tile([128, 24], F32)
        A1 = const.tile([128, 24], F32)
        s2acc = const.tile([128, C], F32)
        s2b = const.tile([128, C], F32)
        s2col = const.tile([128, 6], F32)
        i2col = const.tile([128, 6], F32)
        es2 = const.tile([128, 6], F32)
        invs2_bc = const.tile([128, C], F32)

        # w2 stream rings at the bottom of the left stack: persist from fc1
        # through fc2's first chunk, below gemm1 so gemm1 can close first
        w2s1 = tc.tile_pool(name="w2s1", bufs=2)
        w2s1p = w2s1.__enter__()
        w2m = tc.tile_pool(name="w2m", bufs=5)
        w2mp = w2m.__enter__()

        gemm1 = ExitStack()
        xqp = gemm1.enter_context(tc.tile_pool(name="xqp", bufs=1))
        w1qp = gemm1.enter_context(tc.tile_pool(name="w1qp", bufs=1))
        xq = [xqp.tile([128, t_cov], BF16, name=f"xq{i}") for i in range(6)]
        w1q = [w1qp.tile([128, H], BF16, name=f"w1q{i}") for i in range(6)]

        # tail pools (open through fc1, below the transient x pools)
        w1p2 = tc.tile_pool(name="w1p2", bufs=6)
        w1r = w1p2.__enter__()
        s1sc = tc.tile_pool(name="s1sc", bufs=1)
        s1p = s1sc.__enter__()
        invs1_bc = s1p.tile([128, H], F32, name="invs1bc")
        wqsc = tc.tile_pool(name="wqsc", bufs=3)
        wqp_ = wqsc.__enter__()

        # ============ Phase A: x load (resident) + stats + AR1 ==================
        xfp = tc.tile_pool(name="xfp", bufs=1)
        xfpp = xfp.__enter__()
        xf = [xfpp.tile([128, t_cov], F32, name=f"xf{i}") for i in range(6)]

        xload_insts = []
        for ct in range(6):
            xload_insts.append(
                SYNC.dma_start(out=xf[ct][:],
                               in_=xT[ct * 128:(ct + 1) * 128, 0:t_cov]))
        # all 12 stats reduces first so the AR1 pack fires ASAP
        for ct in range(6):
            DVE.tensor_reduce(out=stats12[:, ct:ct + 1], in_=xf[ct][:], axis=AX,
                              op=AT.max)
            DVE.tensor_reduce(out=stats12[:, 6 + ct:7 + ct], in_=xf[ct][:], axis=AX,
                              op=AT.min, negate=True)
        pack = SYNC.dma_start(out=st_in[:], in_=stats12[:])
        GPS.collective_compute("AllReduce", AT.max, replica_groups=RG,
                               ins=[st_in.opt()], outs=[st_out.opt()])
        ACT.dma_start(out=stats12[:], in_=st_out[:])
        DVE.tensor_tensor(out=stat_abs[:], in0=stat_max[:], in1=stat_nm[:],
                          op=AT.max)

        # ---- w1 full-row pass for wcol (contiguous rows, full bandwidth;
        # reduces run under the AR1 wait) ----
        w1p1 = tc.tile_pool(name="w1p1", bufs=2)
        w1p1p = w1p1.__enter__()
        for ct in range(6):
            wt = w1p1p.tile([128, H], F32, tag="w1a")
            wl = SYNC.dma_start(out=wt[:], in_=w1T[ct * 128:(ct + 1) * 128, :])
            if ct == 0:
                for xl in xload_insts:
                    add_dep_helper(wl.ins, xl.ins, reason="x DMA priority")
                add_dep_helper(wl.ins, pack.ins,
                               reason="AR1 pack before w1 stream")
            DVE.tensor_reduce(out=wcol[:, ct:ct + 1], in_=wt[:], axis=AX,
                              op=AT.max, apply_absolute_value=True)
        w1p1.__exit__(None, None, None)

        # ---- channel scale cs = pow2-snap(sqrt(gmax/wmax)) ----
        rw = const.tile([128, 6], F32)
        DVE.reciprocal(out=rw[:], in_=wcol[:])
        ratio = const.tile([128, 6], F32)
        DVE.tensor_tensor(out=ratio[:], in0=stat_abs[:], in1=rw[:], op=AT.mult)
        yf = const.tile([128, 6], F32)
        ACT.activation(out=yf[:], in_=ratio[:], func=AFT.Ln)
        DVE.tensor_scalar(out=yf[:], in0=yf[:], scalar1=0.5 * INV_LN2,
                          scalar2=0.5, op0=AT.mult, op1=AT.subtract)
        DVE.tensor_scalar(out=yf[:], in0=yf[:], scalar1=RND, scalar2=RND,
                          op0=AT.add, op1=AT.subtract)
        p2 = const.tile([128, 6], F32)
        ACT.activation(out=p2[:], in_=yf[:], func=AFT.Exp, scale=LN2)
        DVE.tensor_scalar(out=p2[:], in0=p2[:], scalar1=4096.0, scalar2=RND,
                          op0=AT.mult, op1=AT.add)
        DVE.tensor_scalar(out=p2[:], in0=p2[:], scalar1=RND,
                          scalar2=1.0 / 4096.0, op0=AT.subtract, op1=AT.mult)
        ph = const.tile([128, 6], F32)
        DVE.tensor_tensor(out=ph[:], in0=p2[:], in1=p2[:], op=AT.mult)
        DVE.tensor_scalar(out=ph[:], in0=ph[:], scalar1=2.25, scalar2=None,
                          op0=AT.mult)
        upf = const.tile([128, 6], F32)
        DVE.tensor_tensor(out=upf[:], in0=ph[:], in1=ratio[:], op=AT.is_lt)
        up1 = const.tile([128, 6], F32)
        DVE.tensor_scalar(out=up1[:], in0=upf[:], scalar1=1.0, scalar2=None,
                          op0=AT.add)
        cs_pow = const.tile([128, 6], F32)
        DVE.tensor_tensor(out=cs_pow[:], in0=p2[:], in1=up1[:], op=AT.mult)
        ncs_pow = const.tile([128, 6], F32)
        DVE.tensor_scalar(out=ncs_pow[:], in0=cs_pow[:], scalar1=-1.0,
                          scalar2=None, op0=AT.mult)
        yu = const.tile([128, 6], F32)
        DVE.tensor_tensor(out=yu[:], in0=yf[:], in1=upf[:], op=AT.add)
        inv_cs = const.tile([128, 6], F32)
        ACT.activation(out=inv_cs[:], in_=yu[:], func=AFT.Exp, scale=-LN2)
        DVE.tensor_scalar(out=inv_cs[:], in0=inv_cs[:], scalar1=4096.0,
                          scalar2=RND, op0=AT.mult, op1=AT.add)
        DVE.tensor_scalar(out=inv_cs[:], in0=inv_cs[:], scalar1=RND,
                          scalar2=1.0 / 4096.0, op0=AT.subtract, op1=AT.mult)

        # ---- x quant range: both partition reduces in one PAR ----
        t6 = const.tile([128, 6], F32)
        t2 = const.tile([128, 2], F32)
        x2r = const.tile([128, 2], F32)
        DVE.tensor_tensor(out=t6[:], in0=stat_max[:], in1=inv_cs[:], op=AT.mult)
        DVE.tensor_reduce(out=t2[:, 0:1], in_=t6[:], axis=AX, op=AT.max)
        t6b = const.tile([128, 6], F32)
        DVE.tensor_tensor(out=t6b[:], in0=stat_nm[:], in1=inv_cs[:], op=AT.mult)
        DVE.tensor_reduce(out=t2[:, 1:2], in_=t6b[:], axis=AX, op=AT.max)
        GPS.partition_all_reduce(x2r[:], t2[:], channels=128, reduce_op=ROP.max)
        DVE.tensor_scalar(out=x2r[:], in0=x2r[:], scalar1=0.0, scalar2=None,
                          op0=AT.max)
        sx = const.tile([128, 1], F32)
        DVE.tensor_tensor(out=sx[:], in0=x2r[:, 0:1], in1=x2r[:, 1:2], op=AT.add)
        DVE.tensor_scalar(out=sx[:], in0=sx[:], scalar1=R255, scalar2=EPS,
                          op0=AT.mult, op1=AT.max)
        inv_sx = const.tile([128, 1], F32)
        DVE.reciprocal(out=inv_sx[:], in_=sx[:])
        a_x = const.tile([128, 6], F32)
        DVE.tensor_scalar(out=a_x[:], in0=inv_cs[:], scalar1=inv_sx[:, 0:1],
                          scalar2=None, op0=AT.mult)

        # ============ Phase B: quarter tails (pass B stream) + x quant ==========
        xts = tc.tile_pool(name="xts", bufs=2)
        xtp = xts.__enter__()

        def q_tail(q, use_act_abs):
            hs = slice(q * HQ, (q + 1) * HQ)
            cols = slice(q * 6, (q + 1) * 6)
            w1qt = []
            for ct in range(6):
                wt = w1r.tile([128, HQ], F32, tag="w1b")
                SYNC.dma_start(out=wt[:], in_=w1T[ct * 128:(ct + 1) * 128, hs])
                w1qt.append(wt)
            s1a = s1p.tile([128, HQ], F32, tag="s1a", name=f"s1a{q}")
            if use_act_abs:
                ACT.activation(out=s1a[:], in_=w1qt[0][:], func=AFT.Abs,
                               scale=cs_pow[:, 0:1])
                for ct in range(1, 6):
                    at = wqp_.tile([128, HQ], F32, tag="wq", name=f"at{q}_{ct}")
                    ACT.activation(out=at[:], in_=w1qt[ct][:], func=AFT.Abs,
                                   scale=cs_pow[:, ct:ct + 1])
                    DVE.tensor_tensor(out=s1a[:], in0=s1a[:], in1=at[:],
                                      op=AT.max)
            else:
                # double signed-max: max over ct of max(w*cs, w*(-cs))
                DVE.tensor_scalar(out=s1a[:], in0=w1qt[0][:],
                                  scalar1=cs_pow[:, 0:1], scalar2=None,
                                  op0=AT.mult)
                DVE.scalar_tensor_tensor(out=s1a[:], in0=w1qt[0][:],
                                         scalar=ncs_pow[:, 0:1], in1=s1a[:],
                                         op0=AT.mult, op1=AT.max)
                for ct in range(1, 6):
                    DVE.scalar_tensor_tensor(out=s1a[:], in0=w1qt[ct][:],
                                             scalar=cs_pow[:, ct:ct + 1],
                                             in1=s1a[:], op0=AT.mult, op1=AT.max)
                    DVE.scalar_tensor_tensor(out=s1a[:], in0=w1qt[ct][:],
                                             scalar=ncs_pow[:, ct:ct + 1],
                                             in1=s1a[:], op0=AT.mult, op1=AT.max)
            s1b = s1p.tile([128, HQ], F32, tag="s1b", name=f"s1b{q}")
            GPS.partition_all_reduce(s1b[:], s1a[:], channels=128,
                                     reduce_op=ROP.max)
            DVE.tensor_scalar(out=s1b[:], in0=s1b[:], scalar1=R127, scalar2=EPS,
                              op0=AT.mult, op1=AT.max)
            # A1 column form rides the idle PE DMA queue (off critical path)
            ACT.dma_start(out=s1row_d[0:1, hs], in_=s1b[0:1, :])
            ACT.dma_start(
                out=s1col[:, cols],
                in_=s1row_d[0:1, hs].rearrange("a (k p) -> (a p) k", p=128))
            DVE.tensor_scalar(out=A1[:, cols], in0=s1col[:, cols],
                              scalar1=sx[:, 0:1], scalar2=None, op0=AT.mult)
            # invs1 on the PAR-broadcast row form (~2ulp)
            inv1 = s1p.tile([128, HQ], F32, tag="inv1", name=f"inv1{q}")
            scr = wqp_.tile([128, HQ], F32, tag="wq", name=f"scr{q}")
            DVE.reciprocal_approx_accurate(out=inv1[:], in_=s1b[:], scratch=scr[:])
            for ct in range(6):
                wq_ = wqp_.tile([128, HQ], F32, tag="wq")
                DVE.scalar_tensor_tensor(out=wq_[:], in0=w1qt[ct][:],
                                         scalar=cs_pow[:, ct:ct + 1],
                                         in1=inv1[:], op0=AT.mult,
                                         op1=AT.mult)
                DVE.tensor_scalar(out=w1q[ct][:, hs], in0=wq_[:], scalar1=RND,
                                  scalar2=RND, op0=AT.add, op1=AT.subtract)

        q_tail(0, use_act_abs=True)
        # x quant on ACT (x resident; 2 ACT ops per ct)
        for ct in range(6):
            xs2 = xtp.tile([128, t_cov], F32, tag="xts")
            ACT.activation(out=xs2[:], in_=xf[ct][:],
                           func=AFT.Copy, scale=a_x[:, ct:ct + 1], bias=RND)
            ACT.activation(out=xq[ct][:], in_=xs2[:],
                           func=AFT.Copy, scale=1.0, bias=-RND)

        xts.__exit__(None, None, None)
        xfp.__exit__(None, None, None)

        # ============ FC1 + GELU; tails q1-3 + w2 streams interleaved ===========
        hp = ctx.enter_context(tc.tile_pool(name="hp", bufs=1, side="right"))
        h_res = [hp.tile([128, t_cov], F32, name=f"h{i}") for i in range(NRES)]

        w2load_insts = []
        DVE.memset(s2acc[:], 0.0)

        with tc.tile_pool(name="hring", bufs=3) as hring:
            for ht in range(24):
                if ht == 0:
                    q_tail(1, use_act_abs=True)
                elif ht == 2:
                    q_tail(2, use_act_abs=True)
                elif ht == 4:
                    q_tail(3, use_act_abs=True)
                pst01 = psp.tile([128, 1024], F32, tag="ps", name=f"psA_{ht}")
                pst23 = psp2.tile([128, 576], F32, tag="ps2", name=f"psB_{ht}")
                lhs = lambda ct: w1q[ct][:, ht * 128:(ht + 1) * 128]
                for ct in range(6):
                    MM(pst01[:, 0:512], lhsT=lhs(ct), rhs=xq[ct][:, 0:512],
                       start=(ct == 0), stop=(ct == 5), skip_group_check=True)
                    MM(pst01[:, 512:1024], lhsT=lhs(ct), rhs=xq[ct][:, 512:1024],
                       start=(ct == 0), stop=(ct == 5), skip_group_check=True)
                    MM(pst23[:, 0:512], lhsT=lhs(ct), rhs=xq[ct][:, 1024:1536],
                       start=(ct == 0), stop=(ct == 5), skip_group_check=True)
                    MM(pst23[:, 512:576], lhsT=lhs(ct), rhs=xq[ct][:, 1536:1600],
                       start=(ct == 0), stop=(ct == 5), skip_group_check=True)
                if ht < NRES:
                    ACT.activation(out=h_res[ht][:, 0:1024], in_=pst01[:],
                                   func=AFT.Gelu, scale=A1[:, ht:ht + 1],
                                   bias=b1t[:, ht:ht + 1])
                    ACT.activation(out=h_res[ht][:, 1024:1600], in_=pst23[:],
                                   func=AFT.Gelu, scale=A1[:, ht:ht + 1],
                                   bias=b1t[:, ht:ht + 1])
                    DVE.tensor_reduce(out=habs_cols[:, ht:ht + 1],
                                      in_=h_res[ht][:, 0:t_loc], axis=AX,
                                      op=AT.max, apply_absolute_value=True)
                else:
                    idx = ht - NRES
                    k = NRES + 2 * idx
                    hr1 = hring.tile([128, 1024], F32, tag="hr1")
                    ACT.activation(out=hr1[:], in_=pst01[:], func=AFT.Gelu,
                                   scale=A1[:, ht:ht + 1], bias=b1t[:, ht:ht + 1])
                    DVE.tensor_reduce(out=habs_cols[:, k:k + 1],
                                      in_=hr1[:], axis=AX, op=AT.max,
                                      apply_absolute_value=True)
                    SYNC.dma_start(out=hd[idx, :, 0:1024], in_=hr1[:])
                    hr2 = hring.tile([128, 1024], F32, tag="hr1", name=f"hr2_{ht}")
                    ACT.activation(out=hr2[:, 0:576], in_=pst23[:], func=AFT.Gelu,
                                   scale=A1[:, ht:ht + 1], bias=b1t[:, ht:ht + 1])
                    wv = t_loc - 1024  # 552 valid of 576
                    DVE.tensor_reduce(out=habs_cols[:, k + 1:k + 2],
                                      in_=hr2[:, :wv], axis=AX, op=AT.max,
                                      apply_absolute_value=True)
                    SYNC.dma_start(out=hd[idx, :, 1024:1600], in_=hr2[:, 0:576])
                # interleaved w2 absmax stream: GPS squares + DVE max
                if 6 <= ht < 18:
                    for j in range(2):
                        kt = (ht - 6) * 2 + j
                        wt = w2s1p.tile([128, C], F32, tag="w2a")
                        wl = SYNC.dma_start(out=wt[:],
                                            in_=w2T[kt * 128:(kt + 1) * 128, :])
                        w2load_insts.append(wl)
                        GPS.tensor_tensor(out=wt[:], in0=wt[:], in1=wt[:],
                                          op=AT.mult)
                        DVE.tensor_tensor(out=s2acc[:], in0=wt[:], in1=s2acc[:],
                                          op=AT.max)
                elif ht == 18:
                    # s2 finalize on squares; sqrt on the tiny column form
                    GPS.partition_all_reduce(s2b[:], s2acc[:], channels=128,
                                             reduce_op=ROP.max)
                    GPS.dma_start(out=s2row[:], in_=s2b[0:1, :])
                    GPS.dma_start(
                        out=s2col[:],
                        in_=s2row[0:1, :].rearrange("a (k p) -> (a p) k", p=128))
                    ACT.activation(out=s2col[:], in_=s2col[:], func=AFT.Sqrt)
                    DVE.tensor_scalar(out=s2col[:], in0=s2col[:], scalar1=R127,
                                      scalar2=EPS, op0=AT.mult, op1=AT.max)
                    DVE.reciprocal(out=i2col[:], in_=s2col[:])
                    GPS.dma_start(
                        out=i2row[0:1, :].rearrange("a (k p) -> (a p) k", p=128),
                        in_=i2col[:])
                    GPS.dma_start(out=invs2_bc[:],
                                  in_=i2row[0:1, :].to_broadcast([128, C]))

        wqsc.__exit__(None, None, None)
        s1sc.__exit__(None, None, None)
        w1p2.__exit__(None, None, None)

        # ================= AR2 pack (h absmax AllReduce) =================
        hb1 = const.tile([128, 1], F32)
        DVE.tensor_reduce(out=hb1[:], in_=habs_cols[:], axis=AX, op=AT.max)
        habs_r = const.tile([128, 1], F32)
        GPS.partition_all_reduce(habs_r[:], hb1[:], channels=128, reduce_op=ROP.max)
        sc_a = const.tile([1, 8], F32)
        DVE.memset(sc_a[:], 0.0)
        DVE.tensor_copy(out=sc_a[0:1, 0:1], in_=habs_r[0:1, 0:1])
        ACT.dma_start(out=sc_in[:], in_=sc_a[:])
        GPS.collective_compute("AllReduce", AT.max, replica_groups=RG,
                               ins=[sc_in.opt()], outs=[sc_out.opt()])

        # -- w2 quant on DVE: kt 0-11 drain into the AR2 wait --
        gemm1.close()  # frees xq, w1q
        wqp = ctx.enter_context(tc.tile_pool(name="wqp", bufs=1, side="right"))
        w2q = [wqp.tile([128, C], BF16, name=f"w2q{i}") for i in range(24)]

        def w2_quant(kt):
            wt = w2mp.tile([128, C], F32, tag="w2m")
            wl = SYNC.dma_start(out=wt[:], in_=w2T[kt * 128:(kt + 1) * 128, :])
            if kt == 0:
                for pl in w2load_insts:
                    add_dep_helper(wl.ins, pl.ins,
                                   reason="w2 absmax stream priority")
            DVE.tensor_tensor(out=wt[:], in0=wt[:], in1=invs2_bc[:], op=AT.mult)
            DVE.tensor_scalar(out=w2q[kt][:], in0=wt[:], scalar1=RND,
                              scalar2=RND, op0=AT.add, op1=AT.subtract)

        for kt in range(12):
            w2_quant(kt)

        # ================= AR2 unpack -> s_h, es2 =================
        s_h = const.tile([128, 1], F32)
        ACT.dma_start(out=s_h[:], in_=sc_out[0:1, 0:1].to_broadcast([128, 1]))
        DVE.tensor_scalar(out=s_h[:], in0=s_h[:], scalar1=R127, scalar2=EPS,
                          op0=AT.mult, op1=AT.max)
        inv_sh = const.tile([128, 1], F32)
        DVE.reciprocal(out=inv_sh[:], in_=s_h[:])
        DVE.tensor_scalar(out=es2[:], in0=s2col[:], scalar1=s_h[:, 0:1],
                          scalar2=None, op0=AT.mult)

        # ======== FC2 (out in [C, tokens] layout), h quantized just-in-time ======
        outp = ctx.enter_context(tc.tile_pool(name="outp", bufs=1, side="right"))
        out_t = [outp.tile([128, t_cov], F32, name=f"o{i}") for i in range(6)]
        with tc.tile_pool(name="ringA", bufs=3) as ringA, \
             tc.tile_pool(name="tmpp", bufs=3) as tmpp, \
             tc.tile_pool(name="hqB", bufs=4) as hqB:
            for ci, (off, w) in enumerate(CH):
                pst = [psp.tile([128, 1024], F32, tag="ps", name=f"ps2a_{ci}"),
                       psp.tile([128, 1024], F32, tag="ps", name=f"ps2b_{ci}"),
                       psp2.tile([128, 1024], F32, tag="ps2", name=f"ps2c_{ci}")]
                for ht in range(24):
                    if ci == 0 and ht < 12:
                        w2_quant(12 + ht)
                    if ht < NRES:
                        src = h_res[ht][:, off:off + w]
                    else:
                        ra = ringA.tile([128, 512], F32, tag="ra")
                        SYNC.dma_start(out=ra[:, :w],
                                       in_=hd[ht - NRES, :, off:off + w])
                        src = ra[:, :w]
                    tq = tmpp.tile([128, 512], F32, tag="tq")
                    hq = hqB.tile([128, 512], BF16, tag="hq")
                    if (ci * 24 + ht) % 4 == 3:
                        # DVE path: TS(mult,add) then TS(sub) -> bf16
                        DVE.tensor_scalar(out=tq[:, :w], in0=src,
                                          scalar1=inv_sh[:, 0:1], scalar2=RND,
                                          op0=AT.mult, op1=AT.add)
                    else:
                        ACT.activation(out=tq[:, :w], in_=src, func=AFT.Copy,
                                       scale=inv_sh[:, 0:1], bias=RND)
                    DVE.tensor_scalar(out=hq[:, :w], in0=tq[:, :w],
                                      scalar1=RND, scalar2=None, op0=AT.subtract)
                    for cs_ in range(6):
                        b = cs_ // 2
                        o2 = (cs_ % 2) * 512
                        MM(pst[b][:, o2:o2 + w],
                           lhsT=w2q[ht][:, cs_ * 128:(cs_ + 1) * 128],
                           rhs=hq[:, :w], start=(ht == 0), stop=(ht == 23),
                           skip_group_check=True)
                wv = max(0, min(w, t_loc - off))
                for cs_ in range(6):
                    b = cs_ // 2
                    o2 = (cs_ % 2) * 512
                    ACT.activation(out=out_t[cs_][:, off:off + w],
                                   in_=pst[b][:, o2:o2 + w], func=AFT.Identity,
                                   scale=es2[:, cs_:cs_ + 1],
                                   bias=b2t[:, cs_:cs_ + 1])
                    k = ci * 6 + cs_
                    DVE.tensor_reduce(out=omax_cols[:, k:k + 1],
                                      in_=out_t[cs_][:, off:off + wv], axis=AX,
                                      op=AT.max)
                    DVE.tensor_reduce(out=onm_cols[:, k:k + 1],
                                      in_=out_t[cs_][:, off:off + wv], axis=AX,
                                      op=AT.min, negate=True)
        w2m.__exit__(None, None, None)
        w2s1.__exit__(None, None, None)

        # ================= out min/max AllReduce -> final quant =================
        om1 = const.tile([128, 1], F32)
        DVE.tensor_reduce(out=om1[:], in_=omax_cols[:], axis=AX, op=AT.max)
        on1 = const.tile([128, 1], F32)
        DVE.tensor_reduce(out=on1[:], in_=onm_cols[:], axis=AX, op=AT.max)
        o2c = const.tile([128, 2], F32)
        DVE.tensor_copy(out=o2c[:, 0:1], in_=om1[:])
        DVE.tensor_copy(out=o2c[:, 1:2], in_=on1[:])
        o2r = const.tile([128, 2], F32)
        GPS.partition_all_reduce(o2r[:], o2c[:], channels=128, reduce_op=ROP.max)
        sc_b = const.tile([1, 8], F32)
        DVE.memset(sc_b[:], 0.0)
        DVE.tensor_copy(out=sc_b[0:1, 0:2], in_=o2r[0:1, 0:2])
        ACT.dma_start(out=sc_in2[:], in_=sc_b[:])
        GPS.collective_compute("AllReduce", AT.max, replica_groups=RG,
                               ins=[sc_in2.opt()], outs=[sc_out2.opt()])
        oc2 = const.tile([128, 2], F32)
        ACT.dma_start(out=oc2[:], in_=sc_out2[0:1, 0:2].to_broadcast([128, 2]))
        DVE.tensor_scalar(out=oc2[:], in0=oc2[:], scalar1=0.0, scalar2=None,
                          op0=AT.max)
        so = const.tile([128, 1], F32)
        DVE.tensor_tensor(out=so[:], in0=oc2[:, 0:1], in1=oc2[:, 1:2], op=AT.add)
        DVE.tensor_scalar(out=so[:], in0=so[:], scalar1=R255, scalar2=EPS,
                          op0=AT.mult, op1=AT.max)
        inv_so = const.tile([128, 1], F32)
        DVE.reciprocal(out=inv_so[:], in_=so[:])

        # final fake-quant: so*round(out/so)
        with tc.tile_pool(name="of", bufs=3) as ofp:
            for cs_ in range(6):
                ot = ofp.tile([128, t_cov], F32, tag="of")
                ACT.activation(out=ot[:], in_=out_t[cs_][:],
                               func=AFT.Copy, scale=inv_so[:, 0:1], bias=RND)
                DVE.tensor_scalar(out=ot[:], in0=ot[:],
                                  scalar1=RND, scalar2=so[:, 0:1],
                                  op0=AT.subtract, op1=AT.mult)
                SYNC.dma_start(out=out_e[cs_ * 128:(cs_ + 1) * 128, 0:t_cov],
                               in_=ot[:])


_NC_CACHE = {}


def _get_nc(n_cores=N_CORES, t_loc=TLOC):
    key = (n_cores, t_loc)
    if key not in _NC_CACHE:
        _NC_CACHE[key] = build(n_cores, t_loc)
    return _NC_CACHE[key]


def _prep_in_maps(x, w1, b1, w2, b2, n_cores=N_CORES):
    t_loc = x.reshape(-1, C).shape[0] // n_cores
    t_pad = ((t_loc + 127) // 128) * 128
    xf = np.ascontiguousarray(x, dtype=np.float32).reshape(-1, C)
    xT_full = xf.T  # [C, TOK]
    w1 = np.ascontiguousarray(w1, dtype=np.float32)
    w2 = np.ascontiguousarray(w2, dtype=np.float32)
    w1T = np.ascontiguousarray(w1.T)
    w2T = np.ascontiguousarray(w2.T)
    b1 = np.ascontiguousarray(b1, dtype=np.float32)
    b2 = np.ascontiguousarray(b2, dtype=np.float32)
    in_maps = []
    for c in range(n_cores):
        sh = np.zeros((C, t_pad), dtype=np.float32)
        sh[:, :t_loc] = xT_full[:, c * t_loc:(c + 1) * t_loc]
        in_maps.append(dict(xT=sh, w1T=w1T, w2T=w2T, b1=b1, b2=b2))
    return in_maps, t_loc


def _install_profile_hook():
    """Provide the antenv.axon_hooks shim this image lacks, so trace=True can
    capture NTFF profiles through libaxon_pjrt."""
    import types
    if "antenv.axon_hooks" in sys.modules:
        return True
    try:
        import antenv
        mod = types.ModuleType("antenv.axon_hooks")
        holder = {}
        mod.set_axon_ntff_profile_hook = lambda h: holder.__setitem__("v", h)
        mod.get_axon_ntff_profile_hook = lambda: holder.get("v")
        sys.modules["antenv.axon_hooks"] = mod
        antenv.axon_hooks = mod
        from trn_agent_boot.trn_boot import _ntff_profile_via_ctypes
        mod.set_axon_ntff_profile_hook(
            _ntff_profile_via_ctypes("/opt/axon/libaxon_pjrt.so"))
        return True
    except Exception as e:  # profiling is best-effort
        print(f"[kernel] profile hook install failed: {e}")
        return False


def kernel(x, w1, b1, w2, b2, trace=False):
    from concourse.bass_utils import run_bass_kernel_spmd

    if trace:
        trace = _install_profile_hook()

    x = np.asarray(x)
    in_maps, t_loc = _prep_in_maps(x, w1, b1, w2, b2)
    nc = _get_nc(N_CORES, t_loc)
    res = run_bass_kernel_spmd(nc, in_maps, core_ids=list(range(N_CORES)),
                               trace=trace)
    out = np.concatenate(
        [np.ascontiguousarray(res.results[c]["out"][:, :t_loc].T)
         for c in range(N_CORES)], axis=0)
    out = out.reshape(x.shape).astype(np.float32)
    kernel.last_results = res
    return out


# revision 28
# speedup vs baseline: 1.0399x; 1.0399x over previous
"""Trainium2 Bass kernel for nn_Mlp_84275848282705 (SmoothQuant-style quantized ViT MLP).

v5: data-parallel over tokens (12608 = 8 x 1576).
- x SBUF-resident through quant (no re-read stream).
- w1 streamed twice in H-quarter column tiles: pass A feeds the wcol
  (per-channel absmax) reduces only; pass B (emitted per-quarter, ring)
  feeds s1 + quant. No full-row pass, no 9 MB residency.
- cs chain: y = round(0.5*log2(ratio) - 0.5); up-compare on squares.
- sx chain: both partition reduces in ONE PartitionAllReduce ([128,2]).
- s1 scale/recip on the tiny [128,6] column form (DMA bounce on GpSimd's
  queue); A1 and invs1_bc derive from it.
- s1 absmax: ACT Abs for q0/q1 (ACT is idle early), double signed-max
  STT chains on DVE for q2/q3 (ACT saturates with gelu later).
- Quarter tails q1..q3 are emitted inside the fc1 loop so the ACT queue
  interleaves abs with gelu instead of blocking gelu behind them.
- fc1 PSUM paired: one [128,1024] psum tile holds chunks 0+1 (two accum
  groups), one [128,576] holds chunks 2+3; gelu reads each pair in a
  single ACT op (halves gelu op count).
- w2 absmax stream: GpSimd squares (TT mult) + DVE max; sqrt on the
  tiny column form.
- w2 quant on DVE (TT mult + TS round): kt 0-11 between AR2 pack and
  unpack, kt 12-23 interleaved into fc2's first chunk.
- fc2 hq: round-trick split between ACT and DVE to balance both queues.
"""
import sys

sys.path.insert(0, "/opt/trn_rl_repo")

import numpy as np

B, N, C, H = 64, 197, 768, 3072
TOK = B * N             # 12608
N_CORES = 8
TLOC = TOK // N_CORES   # 1576
NRES = 11               # h tiles resident in SBUF (of 24); last 13 spill to DRAM
RND = 12582912.0        # 1.5*2^23: RNE integer-round magic const (valid for |x| <= 2^22)
EPS = 1e-8
INV_LN2 = 1.4426950408889634
LN2 = 0.6931471805599453
R127 = float(np.float32(1.0) / np.float32(127.0))
R255 = float(np.float32(1.0) / np.float32(255.0))
HQ = H // 4             # 768: H quarter


def _chunks(t_pad, step, t_loc):
    t_cov = ((t_loc + 63) // 64) * 64
    out, off = [], 0
    while off < t_cov:
        w = min(step, t_cov - off)
        out.append((off, w))
        off += w
    return out


def build(n_cores=N_CORES, t_loc=TLOC):
    import concourse.bacc as bacc
    import concourse.tile as tile
    from concourse import mybir

    F32 = mybir.dt.float32
    t_pad = ((t_loc + 127) // 128) * 128

    nc = bacc.Bacc("TRN2", target_bir_lowering=False, debug=False,
                   enable_asserts=False, num_devices=n_cores)

    io = dict(
        xT=nc.dram_tensor("xT", [C, t_pad], F32, kind="ExternalInput").ap(),
        w1T=nc.dram_tensor("w1T", [C, H], F32, kind="ExternalInput").ap(),
        w2T=nc.dram_tensor("w2T", [H, C], F32, kind="ExternalInput").ap(),
        b1=nc.dram_tensor("b1", [H], F32, kind="ExternalInput").ap(),
        b2=nc.dram_tensor("b2", [C], F32, kind="ExternalInput").ap(),
        # transposed output: [C, t_pad]; host transposes back
        out_e=nc.dram_tensor("out", [C, t_pad], F32, kind="ExternalOutput").ap(),
    )

    with tile.TileContext(nc) as tc:
        _emit(nc, tc, io, n_cores, t_loc, t_pad)
    nc.compile()
    return nc


def _emit(nc, tc, io, n_cores, t_loc, t_pad):
    from contextlib import ExitStack
    from concourse import mybir, bass_isa
    from concourse.tile import add_dep_helper

    F32 = mybir.dt.float32
    BF16 = mybir.dt.bfloat16
    AT = mybir.AluOpType
    AFT = mybir.ActivationFunctionType
    AX = mybir.AxisListType.X
    ROP = bass_isa.ReduceOp
    RG = [list(range(n_cores))]

    xT, w1T, w2T, b1, b2, out_e = (io[k] for k in
                                   ("xT", "w1T", "w2T", "b1", "b2", "out_e"))

    CH = _chunks(t_pad, 512, t_loc)
    t_cov = ((t_loc + 63) // 64) * 64  # 1600
    n_spill = 24 - NRES

    DVE, ACT, GPS, SYNC = nc.vector, nc.scalar, nc.gpsimd, nc.sync
    MM = nc.tensor.matmul

    with ExitStack() as ctx:
        const = ctx.enter_context(tc.tile_pool(name="const", bufs=1))
        dram = ctx.enter_context(tc.tile_pool(name="dram", bufs=1, space="DRAM"))
        psp = ctx.enter_context(tc.tile_pool(name="ps", bufs=2, space="PSUM"))
        psp2 = ctx.enter_context(tc.tile_pool(name="ps2", bufs=2, space="PSUM"))

        # DRAM scratch (collectives + layout bounces + h spill)
        st_in = dram.tile([128, 12], F32)
        st_out = dram.tile([128, 12], F32)
        sc_in = dram.tile([1, 8], F32)
        sc_out = dram.tile([1, 8], F32)
        sc_in2 = dram.tile([1, 8], F32)
        sc_out2 = dram.tile([1, 8], F32)
        s1row_d = dram.tile([1, H], F32)
        i1row_d = dram.tile([1, H], F32)
        s2row = dram.tile([1, C], F32)
        i2row = dram.tile([1, C], F32)
        hd = dram.tile([n_spill, 128, t_cov], F32)

        # small const tiles
        b1t = const.tile([128, 24], F32)
        SYNC.dma_start(out=b1t[:], in_=b1.rearrange("(k p) -> p k", p=128))
        b2t = const.tile([128, 6], F32)
        SYNC.dma_start(out=b2t[:], in_=b2.rearrange("(k p) -> p k", p=128))

        stats12 = const.tile([128, 12], F32)
        stat_max = stats12[:, 0:6]
        stat_nm = stats12[:, 6:12]
        stat_abs = const.tile([128, 6], F32)
        wcol = const.tile([128, 6], F32)
        wcol_t = const.tile([128, 24], F32)
        n_hcols = NRES + 2 * n_spill
        habs_cols = const.tile([128, n_hcols], F32)
        omax_cols = const.tile([128, 24], F32)
        onm_cols = const.tile([128, 24], F32)
        s1col = const.tile([128, 24], F32)
        i1col = const.tile([128, 24], F32)
        A1 = const.tile([128, 24], F32)
        s2acc = const.tile([128, C], F32)
        s2b = const.tile([128, C], F32)
        s2col = const.tile([128, 6], F32)
        i2col = const.tile([128, 6], F32)
        es2 = const.tile([128, 6], F32)
        invs2_bc = const.tile([128, C], F32)

        # w2 stream rings at the bottom of the left stack: persist from fc1
        # through fc2's first chunk, below gemm1 so gemm1 can close first
        w2s1 = tc.tile_pool(name="w2s1", bufs=2)
        w2s1p = w2s1.__enter__()
        w2m = tc.tile_pool(name="w2m", bufs=4)
        w2mp = w2m.__enter__()

        gemm1 = ExitStack()
        xqp = gemm1.enter_context(tc.tile_pool(name="xqp", bufs=1))
        w1qp = gemm1.enter_context(tc.tile_pool(name="w1qp", bufs=1))
        xq = [xqp.tile([128, t_cov], BF16, name=f"xq{i}") for i in range(6)]
        w1q = [w1qp.tile([128, H], BF16, name=f"w1q{i}") for i in range(6)]

        # tail pools (open through fc1, below the transient x pools)
        w1p2 = tc.tile_pool(name="w1p2", bufs=6)
        w1r = w1p2.__enter__()
        s1sc = tc.tile_pool(name="s1sc", bufs=1)
        s1p = s1sc.__enter__()
        invs1_bc = s1p.tile([128, H], F32, name="invs1bc")
        wqsc = tc.tile_pool(name="wqsc", bufs=2)
        wqp_ = wqsc.__enter__()

        # ============ Phase A: x load (resident) + stats + AR1 ==================
        xfp = tc.tile_pool(name="xfp", bufs=1)
        xfpp = xfp.__enter__()
        xf = [xfpp.tile([128, t_cov], F32, name=f"xf{i}") for i in range(6)]

        xload_insts = []
        for ct in range(6):
            xload_insts.append(
                SYNC.dma_start(out=xf[ct][:],
                               in_=xT[ct * 128:(ct + 1) * 128, 0:t_cov]))
        # all 12 stats reduces first so the AR1 pack fires ASAP
        for ct in range(6):
            DVE.tensor_reduce(out=stats12[:, ct:ct + 1], in_=xf[ct][:], axis=AX,
                              op=AT.max)
            DVE.tensor_reduce(out=stats12[:, 6 + ct:7 + ct], in_=xf[ct][:], axis=AX,
                              op=AT.min, negate=True)
        pack = SYNC.dma_start(out=st_in[:], in_=stats12[:])
        GPS.collective_compute("AllReduce", AT.max, replica_groups=RG,
                               ins=[st_in.opt()], outs=[st_out.opt()])
        ACT.dma_start(out=stats12[:], in_=st_out[:])
        DVE.tensor_tensor(out=stat_abs[:], in0=stat_max[:], in1=stat_nm[:],
                          op=AT.max)

        # ---- w1 full-row pass for wcol (contiguous rows, full bandwidth;
        # reduces run under the AR1 wait) ----
        w1p1 = tc.tile_pool(name="w1p1", bufs=2)
        w1p1p = w1p1.__enter__()
        for ct in range(6):
            wt = w1p1p.tile([128, H], F32, tag="w1a")
            wl = SYNC.dma_start(out=wt[:], in_=w1T[ct * 128:(ct + 1) * 128, :])
            if ct == 0:
                for xl in xload_insts:
                    add_dep_helper(wl.ins, xl.ins, reason="x DMA priority")
                add_dep_helper(wl.ins, pack.ins,
                               reason="AR1 pack before w1 stream")
            DVE.tensor_reduce(out=wcol[:, ct:ct + 1], in_=wt[:], axis=AX,
                              op=AT.max, apply_absolute_value=True)
        w1p1.__exit__(None, None, None)

        # ---- channel scale cs = pow2-snap(sqrt(gmax/wmax)) ----
        rw = const.tile([128, 6], F32)
        DVE.reciprocal(out=rw[:], in_=wcol[:])
        ratio = const.tile([128, 6], F32)
        DVE.tensor_tensor(out=ratio[:], in0=stat_abs[:], in1=rw[:], op=AT.mult)
        yf = const.tile([128, 6], F32)
        ACT.activation(out=yf[:], in_=ratio[:], func=AFT.Ln)
        DVE.tensor_scalar(out=yf[:], in0=yf[:], scalar1=0.5 * INV_LN2,
                          scalar2=0.5, op0=AT.mult, op1=AT.subtract)
        DVE.tensor_scalar(out=yf[:], in0=yf[:], scalar1=RND, scalar2=RND,
                          op0=AT.add, op1=AT.subtract)
        p2 = const.tile([128, 6], F32)
        ACT.activation(out=p2[:], in_=yf[:], func=AFT.Exp, scale=LN2)
        DVE.tensor_scalar(out=p2[:], in0=p2[:], scalar1=4096.0, scalar2=RND,
                          op0=AT.mult, op1=AT.add)
        DVE.tensor_scalar(out=p2[:], in0=p2[:], scalar1=RND,
                          scalar2=1.0 / 4096.0, op0=AT.subtract, op1=AT.mult)
        ph = const.tile([128, 6], F32)
        DVE.tensor_tensor(out=ph[:], in0=p2[:], in1=p2[:], op=AT.mult)
        DVE.tensor_scalar(out=ph[:], in0=ph[:], scalar1=2.25, scalar2=None,
                          op0=AT.mult)
        upf = const.tile([128, 6], F32)
        DVE.tensor_tensor(out=upf[:], in0=ph[:], in1=ratio[:], op=AT.is_lt)
        up1 = const.tile([128, 6], F32)
        DVE.tensor_scalar(out=up1[:], in0=upf[:], scalar1=1.0, scalar2=None,
                          op0=AT.add)
        cs_pow = const.tile([128, 6], F32)
        DVE.tensor_tensor(out=cs_pow[:], in0=p2[:], in1=up1[:], op=AT.mult)
        ncs_pow = const.tile([128, 6], F32)
        DVE.tensor_scalar(out=ncs_pow[:], in0=cs_pow[:], scalar1=-1.0,
                          scalar2=None, op0=AT.mult)
        yu = const.tile([128, 6], F32)
        DVE.tensor_tensor(out=yu[:], in0=yf[:], in1=upf[:], op=AT.add)
        inv_cs = const.tile([128, 6], F32)
        ACT.activation(out=inv_cs[:], in_=yu[:], func=AFT.Exp, scale=-LN2)
        DVE.tensor_scalar(out=inv_cs[:], in0=inv_cs[:], scalar1=4096.0,
                          scalar2=RND, op0=AT.mult, op1=AT.add)
        DVE.tensor_scalar(out=inv_cs[:], in0=inv_cs[:], scalar1=RND,
                          scalar2=1.0 / 4096.0, op0=AT.subtract, op1=AT.mult)

        # ---- x quant range: both partition reduces in one PAR ----
        t6 = const.tile([128, 6], F32)
        t2 = const.tile([128, 2], F32)
        x2r = const.tile([128, 2], F32)
        DVE.tensor_tensor(out=t6[:], in0=stat_max[:], in1=inv_cs[:], op=AT.mult)
        DVE.tensor_reduce(out=t2[:, 0:1], in_=t6[:], axis=AX, op=AT.max)
        t6b = const.tile([128, 6], F32)
        DVE.tensor_tensor(out=t6b[:], in0=stat_nm[:], in1=inv_cs[:], op=AT.mult)
        DVE.tensor_reduce(out=t2[:, 1:2], in_=t6b[:], axis=AX, op=AT.max)
        GPS.partition_all_reduce(x2r[:], t2[:], channels=128, reduce_op=ROP.max)
        DVE.tensor_scalar(out=x2r[:], in0=x2r[:], scalar1=0.0, scalar2=None,
                          op0=AT.max)
        sx = const.tile([128, 1], F32)
        DVE.tensor_tensor(out=sx[:], in0=x2r[:, 0:1], in1=x2r[:, 1:2], op=AT.add)
        DVE.tensor_scalar(out=sx[:], in0=sx[:], scalar1=R255, scalar2=EPS,
                          op0=AT.mult, op1=AT.max)
        inv_sx = const.tile([128, 1], F32)
        DVE.reciprocal(out=inv_sx[:], in_=sx[:])
        a_x = const.tile([128, 6], F32)
        DVE.tensor_scalar(out=a_x[:], in0=inv_cs[:], scalar1=inv_sx[:, 0:1],
                          scalar2=None, op0=AT.mult)

        # ============ Phase B: quarter tails (pass B stream) + x quant ==========
        xts = tc.tile_pool(name="xts", bufs=2)
        xtp = xts.__enter__()

        def q_tail(q, use_act_abs):
            hs = slice(q * HQ, (q + 1) * HQ)
            cols = slice(q * 6, (q + 1) * 6)
            w1qt = []
            for ct in range(6):
                wt = w1r.tile([128, HQ], F32, tag="w1b")
                SYNC.dma_start(out=wt[:], in_=w1T[ct * 128:(ct + 1) * 128, hs])
                w1qt.append(wt)
            s1a = s1p.tile([128, HQ], F32, tag="s1a", name=f"s1a{q}")
            if use_act_abs:
                ACT.activation(out=s1a[:], in_=w1qt[0][:], func=AFT.Abs,
                               scale=cs_pow[:, 0:1])
                for ct in range(1, 6):
                    at = wqp_.tile([128, HQ], F32, tag="wq", name=f"at{q}_{ct}")
                    ACT.activation(out=at[:], in_=w1qt[ct][:], func=AFT.Abs,
                                   scale=cs_pow[:, ct:ct + 1])
                    DVE.tensor_tensor(out=s1a[:], in0=s1a[:], in1=at[:],
                                      op=AT.max)
            else:
                # double signed-max: max over ct of max(w*cs, w*(-cs))
                DVE.tensor_scalar(out=s1a[:], in0=w1qt[0][:],
                                  scalar1=cs_pow[:, 0:1], scalar2=None,
                                  op0=AT.mult)
                DVE.scalar_tensor_tensor(out=s1a[:], in0=w1qt[0][:],
                                         scalar=ncs_pow[:, 0:1], in1=s1a[:],
                                         op0=AT.mult, op1=AT.max)
                for ct in range(1, 6):
                    DVE.scalar_tensor_tensor(out=s1a[:], in0=w1qt[ct][:],
                                             scalar=cs_pow[:, ct:ct + 1],
                                             in1=s1a[:], op0=AT.mult, op1=AT.max)
                    DVE.scalar_tensor_tensor(out=s1a[:], in0=w1qt[ct][:],
                                             scalar=ncs_pow[:, ct:ct + 1],
                                             in1=s1a[:], op0=AT.mult, op1=AT.max)
            s1b = s1p.tile([128, HQ], F32, tag="s1b", name=f"s1b{q}")
            GPS.partition_all_reduce(s1b[:], s1a[:], channels=128,
                                     reduce_op=ROP.max)
            DVE.tensor_scalar(out=s1b[:], in0=s1b[:], scalar1=R127, scalar2=EPS,
                              op0=AT.mult, op1=AT.max)
            # A1 column form rides the idle PE DMA queue (off critical path)
            ACT.dma_start(out=s1row_d[0:1, hs], in_=s1b[0:1, :])
            ACT.dma_start(
                out=s1col[:, cols],
                in_=s1row_d[0:1, hs].rearrange("a (k p) -> (a p) k", p=128))
            DVE.tensor_scalar(out=A1[:, cols], in0=s1col[:, cols],
                              scalar1=sx[:, 0:1], scalar2=None, op0=AT.mult)
            # invs1 on the PAR-broadcast row form (~2ulp)
            inv1 = s1p.tile([128, HQ], F32, tag="inv1", name=f"inv1{q}")
            scr = wqp_.tile([128, HQ], F32, tag="wq", name=f"scr{q}")
            DVE.reciprocal_approx_accurate(out=inv1[:], in_=s1b[:], scratch=scr[:])
            for ct in range(6):
                wq_ = wqp_.tile([128, HQ], F32, tag="wq")
                DVE.scalar_tensor_tensor(out=wq_[:], in0=w1qt[ct][:],
                                         scalar=cs_pow[:, ct:ct + 1],
                                         in1=inv1[:], op0=AT.mult,
                                         op1=AT.mult)
                DVE.tensor_scalar(out=w1q[ct][:, hs], in0=wq_[:], scalar1=RND,
                                  scalar2=RND, op0=AT.add, op1=AT.subtract)

        q_tail(0, use_act_abs=True)
        # x quant on ACT (x resident; 2 ACT ops per ct)
        for ct in range(6):
            xs2 = xtp.tile([128, t_cov], F32, tag="xts")
            ACT.activation(out=xs2[:], in_=xf[ct][:],
                           func=AFT.Copy, scale=a_x[:, ct:ct + 1], bias=RND)
            ACT.activation(out=xq[ct][:], in_=xs2[:],
                           func=AFT.Copy, scale=1.0, bias=-RND)

        xts.__exit__(None, None, None)
        xfp.__exit__(None, None, None)

        # ============ FC1 + GELU; tails q1-3 + w2 streams interleaved ===========
        hp = ctx.enter_context(tc.tile_pool(name="hp", bufs=1, side="right"))
        h_res = [hp.tile([128, t_cov], F32, name=f"h{i}") for i in range(NRES)]

        w2load_insts = []
        DVE.memset(s2acc[:], 0.0)

        with tc.tile_pool(name="hring", bufs=2) as hring:
            for ht in range(24):
                if ht == 0:
                    q_tail(1, use_act_abs=True)
                elif ht == 2:
                    q_tail(2, use_act_abs=True)
                elif ht == 4:
                    q_tail(3, use_act_abs=True)
                pst01 = psp.tile([128, 1024], F32, tag="ps", name=f"psA_{ht}")
                pst23 = psp2.tile([128, 576], F32, tag="ps2", name=f"psB_{ht}")
                lhs = lambda ct: w1q[ct][:, ht * 128:(ht + 1) * 128]
                for ct in range(6):
                    MM(pst01[:, 0:512], lhsT=lhs(ct), rhs=xq[ct][:, 0:512],
                       start=(ct == 0), stop=(ct == 5), skip_group_check=True)
                    MM(pst01[:, 512:1024], lhsT=lhs(ct), rhs=xq[ct][:, 512:1024],
                       start=(ct == 0), stop=(ct == 5), skip_group_check=True)
                    MM(pst23[:, 0:512], lhsT=lhs(ct), rhs=xq[ct][:, 1024:1536],
                       start=(ct == 0), stop=(ct == 5), skip_group_check=True)
                    MM(pst23[:, 512:576], lhsT=lhs(ct), rhs=xq[ct][:, 1536:1600],
                       start=(ct == 0), stop=(ct == 5), skip_group_check=True)
                if ht < NRES:
                    ACT.activation(out=h_res[ht][:, 0:1024], in_=pst01[:],
                                   func=AFT.Gelu, scale=A1[:, ht:ht + 1],
                                   bias=b1t[:, ht:ht + 1])
                    ACT.activation(out=h_res[ht][:, 1024:1600], in_=pst23[:],
                                   func=AFT.Gelu, scale=A1[:, ht:ht + 1],
                                   bias=b1t[:, ht:ht + 1])
                    DVE.tensor_reduce(out=habs_cols[:, ht:ht + 1],
                                      in_=h_res[ht][:, 0:t_loc], axis=AX,
                                      op=AT.max, apply_absolute_value=True)
                else:
                    idx = ht - NRES
                    k = NRES + 2 * idx
                    hr1 = hring.tile([128, 1024], F32, tag="hr1")
                    ACT.activation(out=hr1[:], in_=pst01[:], func=AFT.Gelu,
                                   scale=A1[:, ht:ht + 1], bias=b1t[:, ht:ht + 1])
                    DVE.tensor_reduce(out=habs_cols[:, k:k + 1],
                                      in_=hr1[:], axis=AX, op=AT.max,
                                      apply_absolute_value=True)
                    SYNC.dma_start(out=hd[idx, :, 0:1024], in_=hr1[:])
                    hr2 = hring.tile([128, 1024], F32, tag="hr1", name=f"hr2_{ht}")
                    ACT.activation(out=hr2[:, 0:576], in_=pst23[:], func=AFT.Gelu,
                                   scale=A1[:, ht:ht + 1], bias=b1t[:, ht:ht + 1])
                    wv = t_loc - 1024  # 552 valid of 576
                    DVE.tensor_reduce(out=habs_cols[:, k + 1:k + 2],
                                      in_=hr2[:, :wv], axis=AX, op=AT.max,
                                      apply_absolute_value=True)
                    SYNC.dma_start(out=hd[idx, :, 1024:1600], in_=hr2[:, 0:576])
                # interleaved w2 absmax stream: GPS squares + DVE max
                if 6 <= ht < 18:
                    for j in range(2):
                        kt = (ht - 6) * 2 + j
                        wt = w2s1p.tile([128, C], F32, tag="w2a")
                        wl = SYNC.dma_start(out=wt[:],
                                            in_=w2T[kt * 128:(kt + 1) * 128, :])
                        w2load_insts.append(wl)
                        GPS.tensor_tensor(out=wt[:], in0=wt[:], in1=wt[:],
                                          op=AT.mult)
                        DVE.tensor_tensor(out=s2acc[:], in0=wt[:], in1=s2acc[:],
                                          op=AT.max)
                elif ht == 18:
                    # s2 finalize on squares; sqrt on the tiny column form
                    GPS.partition_all_reduce(s2b[:], s2acc[:], channels=128,
                                             reduce_op=ROP.max)
                    GPS.dma_start(out=s2row[:], in_=s2b[0:1, :])
                    GPS.dma_start(
                        out=s2col[:],
                        in_=s2row[0:1, :].rearrange("a (k p) -> (a p) k", p=128))
                    ACT.activation(out=s2col[:], in_=s2col[:], func=AFT.Sqrt)
                    DVE.tensor_scalar(out=s2col[:], in0=s2col[:], scalar1=R127,
                                      scalar2=EPS, op0=AT.mult, op1=AT.max)
                    DVE.reciprocal(out=i2col[:], in_=s2col[:])
                    GPS.dma_start(
                        out=i2row[0:1, :].rearrange("a (k p) -> (a p) k", p=128),
                        in_=i2col[:])
                    GPS.dma_start(out=invs2_bc[:],
                                  in_=i2row[0:1, :].to_broadcast([128, C]))

        wqsc.__exit__(None, None, None)
        s1sc.__exit__(None, None, None)
        w1p2.__exit__(None, None, None)

        # ================= AR2 pack (h absmax AllReduce) =================
        hb1 = const.tile([128, 1], F32)
        DVE.tensor_reduce(out=hb1[:], in_=habs_cols[:], axis=AX, op=AT.max)
        habs_r = const.tile([128, 1], F32)
        GPS.partition_all_reduce(habs_r[:], hb1[:], channels=128, reduce_op=ROP.max)
        sc_a = const.tile([1, 8], F32)
        DVE.memset(sc_a[:], 0.0)
        DVE.tensor_copy(out=sc_a[0:1, 0:1], in_=habs_r[0:1, 0:1])
        ACT.dma_start(out=sc_in[:], in_=sc_a[:])
        GPS.collective_compute("AllReduce", AT.max, replica_groups=RG,
                               ins=[sc_in.opt()], outs=[sc_out.opt()])

        # -- w2 quant on DVE: kt 0-11 drain into the AR2 wait --
        gemm1.close()  # frees xq, w1q
        wqp = ctx.enter_context(tc.tile_pool(name="wqp", bufs=1, side="right"))
        w2q = [wqp.tile([128, C], BF16, name=f"w2q{i}") for i in range(24)]

        def w2_quant(kt):
            wt = w2mp.tile([128, C], F32, tag="w2m")
            wl = SYNC.dma_start(out=wt[:], in_=w2T[kt * 128:(kt + 1) * 128, :])
            if kt == 0:
                for pl in w2load_insts:
                    add_dep_helper(wl.ins, pl.ins,
                                   reason="w2 absmax stream priority")
            DVE.tensor_tensor(out=wt[:], in0=wt[:], in1=invs2_bc[:], op=AT.mult)
            DVE.tensor_scalar(out=w2q[kt][:], in0=wt[:], scalar1=RND,
                              scalar2=RND, op0=AT.add, op1=AT.subtract)

        for kt in range(12):
            w2_quant(kt)

        # ================= AR2 unpack -> s_h, es2 =================
        s_h = const.tile([128, 1], F32)
        ACT.dma_start(out=s_h[:], in_=sc_out[0:1, 0:1].to_broadcast([128, 1]))
        DVE.tensor_scalar(out=s_h[:], in0=s_h[:], scalar1=R127, scalar2=EPS,
                          op0=AT.mult, op1=AT.max)
        inv_sh = const.tile([128, 1], F32)
        DVE.reciprocal(out=inv_sh[:], in_=s_h[:])
        DVE.tensor_scalar(out=es2[:], in0=s2col[:], scalar1=s_h[:, 0:1],
                          scalar2=None, op0=AT.mult)

        # ======== FC2 (out in [C, tokens] layout), h quantized just-in-time ======
        outp = ctx.enter_context(tc.tile_pool(name="outp", bufs=1, side="right"))
        out_t = [outp.tile([128, t_cov], F32, name=f"o{i}") for i in range(6)]
        with tc.tile_pool(name="ringA", bufs=3) as ringA, \
             tc.tile_pool(name="tmpp", bufs=3) as tmpp, \
             tc.tile_pool(name="hqB", bufs=4) as hqB:
            for ci, (off, w) in enumerate(CH):
                pst = [psp.tile([128, 1024], F32, tag="ps", name=f"ps2a_{ci}"),
                       psp.tile([128, 1024], F32, tag="ps", name=f"ps2b_{ci}"),
                       psp2.tile([128, 1024], F32, tag="ps2", name=f"ps2c_{ci}")]
                for ht in range(24):
                    if ci == 0 and ht < 12:
                        w2_quant(12 + ht)
                    if ht < NRES:
                        src = h_res[ht][:, off:off + w]
                    else:
                        ra = ringA.tile([128, 512], F32, tag="ra")
                        SYNC.dma_start(out=ra[:, :w],
                                       in_=hd[ht - NRES, :, off:off + w])
                        src = ra[:, :w]
                    tq = tmpp.tile([128, 512], F32, tag="tq")
                    hq = hqB.tile([128, 512], BF16, tag="hq")
                    if (ci * 24 + ht) % 4 == 3:
                        # DVE path: TS(mult,add) then TS(sub) -> bf16
                        DVE.tensor_scalar(out=tq[:, :w], in0=src,
                                          scalar1=inv_sh[:, 0:1], scalar2=RND,
                                          op0=AT.mult, op1=AT.add)
                    else:
                        ACT.activation(out=tq[:, :w], in_=src, func=AFT.Copy,
                                       scale=inv_sh[:, 0:1], bias=RND)
                    DVE.tensor_scalar(out=hq[:, :w], in0=tq[:, :w],
                                      scalar1=RND, scalar2=None, op0=AT.subtract)
                    for cs_ in range(6):
                        b = cs_ // 2
                        o2 = (cs_ % 2) * 512
                        MM(pst[b][:, o2:o2 + w],
                           lhsT=w2q[ht][:, cs_ * 128:(cs_ + 1) * 128],
                           rhs=hq[:, :w], start=(ht == 0), stop=(ht == 23),
                           skip_group_check=True)
                wv = max(0, min(w, t_loc - off))
                for cs_ in range(6):
                    b = cs_ // 2
                    o2 = (cs_ % 2) * 512
                    ACT.activation(out=out_t[cs_][:, off:off + w],
                                   in_=pst[b][:, o2:o2 + w], func=AFT.Identity,
                                   scale=es2[:, cs_:cs_ + 1],
                                   bias=b2t[:, cs_:cs_ + 1])
                    k = ci * 6 + cs_
                    DVE.tensor_reduce(out=omax_cols[:, k:k + 1],
                                      in_=out_t[cs_][:, off:off + wv], axis=AX,
                                      op=AT.max)
                    DVE.tensor_reduce(out=onm_cols[:, k:k + 1],
                                      in_=out_t[cs_][:, off:off + wv], axis=AX,
                                      op=AT.min, negate=True)
        w2m.__exit__(None, None, None)
        w2s1.__exit__(None, None, None)

        # ================= out min/max AllReduce -> final quant =================
        om1 = const.tile([128, 1], F32)
        DVE.tensor_reduce(out=om1[:], in_=omax_cols[:], axis=AX, op=AT.max)
        on1 = const.tile([128, 1], F32)
        DVE.tensor_reduce(out=on1[:], in_=onm_cols[:], axis=AX, op=AT.max)
        o2c = const.tile([128, 2], F32)
        DVE.tensor_copy(out=o2c[:, 0:1], in_=om1[:])
        DVE.tensor_copy(out=o2c[:, 1:2], in_=on1[:])
        o2r = const.tile([128, 2], F32)
        GPS.partition_all_reduce(o2r[:], o2c[:], channels=128, reduce_op=ROP.max)
        sc_b = const.tile([1, 8], F32)
        DVE.memset(sc_b[:], 0.0)
        DVE.tensor_copy(out=sc_b[0:1, 0:2], in_=o2r[0:1, 0:2])
        ACT.dma_start(out=sc_in2[:], in_=sc_b[:])
        GPS.collective_compute("AllReduce", AT.max, replica_groups=RG,
                               ins=[sc_in2.opt()], outs=[sc_out2.opt()])
        oc2 = const.tile([128, 2], F32)
        ACT.dma_start(out=oc2[:], in_=sc_out2[0:1, 0:2].to_broadcast([128, 2]))
        DVE.tensor_scalar(out=oc2[:], in0=oc2[:], scalar1=0.0, scalar2=None,
                          op0=AT.max)
        so = const.tile([128, 1], F32)
        DVE.tensor_tensor(out=so[:], in0=oc2[:, 0:1], in1=oc2[:, 1:2], op=AT.add)
        DVE.tensor_scalar(out=so[:], in0=so[:], scalar1=R255, scalar2=EPS,
                          op0=AT.mult, op1=AT.max)
        inv_so = const.tile([128, 1], F32)
        DVE.reciprocal(out=inv_so[:], in_=so[:])

        # final fake-quant: so*round(out/so)
        with tc.tile_pool(name="of", bufs=3) as ofp:
            for cs_ in range(6):
                ot = ofp.tile([128, t_cov], F32, tag="of")
                ACT.activation(out=ot[:], in_=out_t[cs_][:],
                               func=AFT.Copy, scale=inv_so[:, 0:1], bias=RND)
                DVE.tensor_scalar(out=ot[:], in0=ot[:],
                                  scalar1=RND, scalar2=so[:, 0:1],
                                  op0=AT.subtract, op1=AT.mult)
                SYNC.dma_start(out=out_e[cs_ * 128:(cs_ + 1) * 128, 0:t_cov],
                               in_=ot[:])


_NC_CACHE = {}


def _get_nc(n_cores=N_CORES, t_loc=TLOC):
    key = (n_cores, t_loc)
    if key not in _NC_CACHE:
        _NC_CACHE[key] = build(n_cores, t_loc)
    return _NC_CACHE[key]


def _prep_in_maps(x, w1, b1, w2, b2, n_cores=N_CORES):
    t_loc = x.reshape(-1, C).shape[0] // n_cores
    t_pad = ((t_loc + 127) // 128) * 128
    xf = np.ascontiguousarray(x, dtype=np.float32).reshape(-1, C)
    xT_full = xf.T  # [C, TOK]
    w1 = np.ascontiguousarray(w1, dtype=np.float32)
    w2 = np.ascontiguousarray(w2, dtype=np.float32)
    w1T = np.ascontiguousarray(w1.T)
    w2T = np.ascontiguousarray(w2.T)
    b1 = np.ascontiguousarray(b1, dtype=np.float32)
    b2 = np.ascontiguousarray(b2, dtype=np.float32)
    in_maps = []
    for c in range(n_cores):
        sh = np.zeros((C, t_pad), dtype=np.float32)
        sh[:, :t_loc] = xT_full[:, c * t_loc:(c + 1) * t_loc]
        in_maps.append(dict(xT=sh, w1T=w1T, w2T=w2T, b1=b1, b2=b2))
    return in_maps, t_loc


def _install_profile_hook():
    """Provide the antenv.axon_hooks shim this image lacks, so trace=True can
    capture NTFF profiles through libaxon_pjrt."""
    import types
    if "antenv.axon_hooks" in sys.modules:
        return True
    try:
        import antenv
        mod = types.ModuleType("antenv.axon_hooks")
        holder = {}
        mod.set_axon_ntff_profile_hook = lambda h: holder.__setitem__("v", h)
        mod.get_axon_ntff_profile_hook = lambda: holder.get("v")
        sys.modules["antenv.axon_hooks"] = mod
        antenv.axon_hooks = mod
        from trn_agent_boot.trn_boot import _ntff_profile_via_ctypes
        mod.set_axon_ntff_profile_hook(
            _ntff_profile_via_ctypes("/opt/axon/libaxon_pjrt.so"))
        return True
    except Exception as e:  # profiling is best-effort
        print(f"[kernel] profile hook install failed: {e}")
        return False


def kernel(x, w1, b1, w2, b2, trace=False):
    from concourse.bass_utils import run_bass_kernel_spmd

    if trace:
        trace = _install_profile_hook()

    x = np.asarray(x)
    in_maps, t_loc = _prep_in_maps(x, w1, b1, w2, b2)
    nc = _get_nc(N_CORES, t_loc)
    res = run_bass_kernel_spmd(nc, in_maps, core_ids=list(range(N_CORES)),
                               trace=trace)
    out = np.concatenate(
        [np.ascontiguousarray(res.results[c]["out"][:, :t_loc].T)
         for c in range(N_CORES)], axis=0)
    out = out.reshape(x.shape).astype(np.float32)
    kernel.last_results = res
    return out
